# revision 1
# baseline (speedup 1.0000x reference)
"""Self-contained Trainium2 Bass kernel for nn_DariushLayer_14087492731059.

kernel(**inputs) takes the FULL unsharded inputs of reference.setup_inputs()
and returns the full [B, S, D] float32 output, computed across 8 NeuronCores:
attention tensor-parallel over heads (2 heads/core), MoE expert-parallel
(1 expert/core), one SPMD launch with an on-device AllReduce at the
attention->MoE boundary.  GEMMs run in fp32r (12-bit-mantissa fp32, 4x rate).
"""

import numpy as np
import concourse.bass as bass
import concourse.tile as tile
from concourse import bacc, mybir
from contextlib import ExitStack

f32, f32r = mybir.dt.float32, mybir.dt.float32r
AF = mybir.ActivationFunctionType
OP = mybir.AluOpType
AX = mybir.AxisListType

B, S, D, H, DK, E = 2, 2048, 1024, 16, 64, 8
T = B * S
NC = 8
KC = D // 128
EPS = 1e-6
MASKNEG = -30000.0


def build_program():
    nc = bacc.Bacc("TRN2", target_bir_lowering=False, debug=False, num_devices=NC)
    dt = nc.dram_tensor
    io = {}
    def inp(nm, shp, ty=f32):
        io[nm] = dt(nm, shp, ty, kind="ExternalInput").ap()
    def outp(nm, shp, ty=f32):
        io[nm] = dt(nm, shp, ty, kind="ExternalOutput").ap()
    inp("x", [T, D])
    for nm in ("wq", "wk", "wv", "wqs", "wks"):
        inp(nm, [D, 128], f32r)
    inp("wo", [128, D], f32r)
    inp("cosb", [128, S]); inp("sinb", [128, S])
    inp("masks", [128, 4, 512])
    inp("ident", [128, 128]); inp("id64", [128, 128])
    inp("rw", [D, E], f32r)
    inp("noise", [T, E])
    inp("sel", [128, E])
    inp("w1", [KC, D, 128], f32r); inp("w2", [KC, D, 128], f32r)
    inp("wout", [D, D], f32r)
    inp("b1h", [128, KC]); inp("b2h", [128, KC])
    outp("out", [T, D])
    outp("h_out", [T, D])

    with tile.TileContext(nc) as tc, ExitStack() as top:
        const = top.enter_context(tc.tile_pool(name="const", bufs=1))
        psum = top.enter_context(tc.tile_pool(name="psum", bufs=8, space="PSUM"))
        dram = top.enter_context(tc.tile_pool(name="dram", bufs=1, space="DRAM"))

        def P(shape=(128, 512)):
            return psum.tile(list(shape), f32, tag="ps", name="ps")

        cst = {}
        for nm, shp in [("ident", [128, 128]), ("id64", [128, 128]),
                        ("sel", [128, E]), ("b1h", [128, KC]), ("b2h", [128, KC])]:
            cst[nm] = const.tile(shp, f32, name=nm)
            nc.sync.dma_start(cst[nm][:], io[nm][:])
        cst["rw"] = const.tile([128, KC, E], f32r, name="rw")
        nc.sync.dma_start(cst["rw"][:], io["rw"].rearrange("(kc p) m -> p kc m", p=128))
        ones = const.tile([128, 1], f32, name="ones")
        nc.vector.memset(ones[:], 1.0)
        eps_t = const.tile([128, 1], f32, name="eps_t")
        nc.vector.memset(eps_t[:], EPS)
        zeros_t = const.tile([128, 1], f32, name="zeros_t")
        nc.vector.memset(zeros_t[:], 0.0)
        wgt_all = const.tile([128, 32], f32, name="wgt_all")

        ar_in = dram.tile([T, D], f32, name="ar_in")
        ar_out = dram.tile([T, D], f32, name="ar_out", addr_space="Shared")

        # --- rmsnorm one [128, D] row-tile and transpose into xT[:, kc, lo] ---
        def norm_transpose(work, src_dram, xT, st, lo, extra=None, src2=None):
            xt = work.tile([128, D], f32, tag="xt", name="xt")
            r0 = st * 128
            nc.sync.dma_start(xt[:], src_dram[r0:r0 + 128, :])
            if src2 is not None:
                a2 = work.tile([128, D], f32, tag="a2", name="a2", bufs=1)
                nc.sync.dma_start(a2[:], src2[r0:r0 + 128, :])
                nc.vector.tensor_tensor(xt[:], xt[:], a2[:], op=OP.add)
            if extra is not None:
                extra(xt)
            sq = work.tile([128, D], f32, tag="xh", name="sq")
            nc.vector.tensor_tensor(sq[:], xt[:], xt[:], op=OP.mult)
            ssum = work.tile([128, 1], f32, tag="ssum", name="ssum")
            nc.vector.reduce_sum(ssum[:], sq[:], axis=AX.X)
            sd = work.tile([128, 1], f32, tag="ssum", name="sd")
            nc.scalar.activation(sd[:], ssum[:], AF.Sqrt, bias=eps_t[:], scale=1.0 / D)
            rr = work.tile([128, 1], f32, tag="ssum", name="rr")
            nc.vector.reciprocal(rr[:], sd[:])
            xh = work.tile([128, D], f32, tag="xh", name="xh")
            nc.vector.tensor_scalar_mul(xh[:], xt[:], rr[:])
            for kc in range(KC):
                pt = P((128, 128))
                nc.tensor.transpose(pt[:], xh[:, kc * 128:(kc + 1) * 128],
                                    cst["ident"][:])
                if kc % 2 == 0:
                    nc.scalar.copy(xT[:, kc, lo:lo + 128], pt[:])
                else:
                    nc.vector.tensor_copy(xT[:, kc, lo:lo + 128], pt[:])

        # =================================================================
        # Phase A: attention
        # =================================================================
        with tc.tile_pool(name="qkv", bufs=1) as qkv, \
             tc.tile_pool(name="apool", bufs=1) as apool:
            qT = qkv.tile([128, T], f32r, name="qT")
            kT = qkv.tile([128, T], f32r, name="kT")
            vT = qkv.tile([128, T], f32, name="vT")
            for nm, shp, ty in [("cosb", [128, S], f32), ("sinb", [128, S], f32),
                                ("masks", [128, 4, 512], f32)]:
                cst[nm] = apool.tile(shp, ty, name=nm)
                nc.sync.dma_start(cst[nm][:], io[nm][:])
            for nm in ("wq", "wk", "wv", "wqs", "wks"):
                cst[nm] = apool.tile([128, KC, 128], f32r, name=nm)
                nc.sync.dma_start(cst[nm][:],
                                  io[nm].rearrange("(kc p) m -> p kc m", p=128))

            with tc.tile_pool(name="xnt", bufs=2) as xnt_pool, \
                 tc.tile_pool(name="work", bufs=2) as work:
                for b in range(B):
                    for sb in range(4):
                        xnT = xnt_pool.tile([128, KC, 512], f32r, tag="xnT",
                                            name="xnT")
                        for q4 in range(4):
                            st = b * 16 + sb * 4 + q4
                            norm_transpose(work, io["x"], xnT, st, q4 * 128)
                        gl = slice(b * S + sb * 512, b * S + (sb + 1) * 512)
                        sl = slice(sb * 512, (sb + 1) * 512)
                        for base, swp, dst in (("wq", "wqs", qT), ("wk", "wks", kT)):
                            pa = P()
                            for kc in range(KC):
                                nc.tensor.matmul(pa[:], cst[base][:, kc, :],
                                                 xnT[:, kc, :],
                                                 start=(kc == 0), stop=(kc == KC - 1))
                            pb = P()
                            for kc in range(KC):
                                nc.tensor.matmul(pb[:], cst[swp][:, kc, :],
                                                 xnT[:, kc, :],
                                                 start=(kc == 0), stop=(kc == KC - 1))
                            t1 = work.tile([128, 512], f32, tag="t1", name="t1")
                            nc.vector.tensor_tensor(t1[:], pa[:], cst["cosb"][:, sl],
                                                    op=OP.mult)
                            t2 = work.tile([128, 512], f32, tag="t2", name="t2")
                            nc.vector.tensor_tensor(t2[:], pb[:], cst["sinb"][:, sl],
                                                    op=OP.mult)
                            nc.vector.tensor_tensor(dst[:, gl], t1[:], t2[:], op=OP.add)
                        pv = P()
                        for kc in range(KC):
                            nc.tensor.matmul(pv[:], cst["wv"][:, kc, :], xnT[:, kc, :],
                                             start=(kc == 0), stop=(kc == KC - 1))
                        nc.scalar.copy(vT[:, gl], pv[:])

            # attention core
            with tc.tile_pool(name="attw", bufs=1) as attw, \
                 tc.tile_pool(name="att", bufs=3) as att, \
                 tc.tile_pool(name="expp", bufs=4) as expp, \
                 tc.tile_pool(name="vsb", bufs=18) as vsbp:
                oT0 = attw.tile([64, T], f32r, name="oT0")
                oT1 = attw.tile([64, T], f32r, name="oT1")
                oTs = [oT0, oT1]
                for b in range(B):
                    for h in range(2):
                        hr = slice(h * 64, (h + 1) * 64)
                        idn = cst["ident"] if h == 0 else cst["id64"]
                        vchunks = []
                        for m in range(16):
                            gk = slice(b * S + m * 128, b * S + (m + 1) * 128)
                            pt = P((128, 64))
                            nc.tensor.transpose(pt[:], vT[hr, gk], idn[hr, 0:64])
                            vs = vsbp.tile([128, 66], f32r, tag="vs", name="vs")
                            nc.scalar.copy(vs[:, 0:64], pt[:])
                            nc.vector.tensor_copy(vs[:, 64:65], ones[:])
                            nc.vector.tensor_copy(vs[:, 65:66], zeros_t[:])
                            vchunks.append(vs)
                        for jq in range(4):
                            gq = slice(b * S + jq * 512, b * S + (jq + 1) * 512)
                            nch = 4 * jq + 4
                            pos = [P((128, 66)) for _ in range(4)]
                            for m in range(nch):
                                gk = slice(b * S + m * 128, b * S + (m + 1) * 128)
                                pse = P()
                                nc.tensor.matmul(pse[:], kT[hr, gk], qT[hr, gq],
                                                 start=True, stop=True)
                                if m >= 4 * jq:
                                    nc.vector.tensor_tensor(
                                        pse[:], pse[:],
                                        cst["masks"][:, m - 4 * jq, :], op=OP.add)
                                et = expp.tile([128, 512], f32r, tag="et", name="et")
                                nc.scalar.activation(et[:], pse[:], AF.Exp, scale=0.125)
                                for qt in range(4):
                                    nc.tensor.matmul(
                                        pos[qt][:], et[:, qt * 128:(qt + 1) * 128],
                                        vchunks[m][:],
                                        start=(m == 0), stop=(m == nch - 1))
                            for qt in range(4):
                                rcp = att.tile([128, 1], f32, tag="rcp", name="rcp")
                                nc.vector.reciprocal(rcp[:], pos[qt][:, 64:65])
                                opr = att.tile([128, 64], f32, tag="opr", name="opr")
                                nc.vector.tensor_scalar_mul(opr[:], pos[qt][:, 0:64],
                                                            rcp[:])
                                ptt = P((128, 128))
                                nc.tensor.transpose(ptt[0:64, :], opr[:],
                                                    cst["ident"][:])
                                g128 = slice(b * S + jq * 512 + qt * 128,
                                             b * S + jq * 512 + (qt + 1) * 128)
                                nc.scalar.copy(oTs[h][:, g128], ptt[0:64, :])

                # out-projection partials -> ar_in
                wo0 = attw.tile([64, D], f32r, name="wo0")
                wo1 = attw.tile([64, D], f32r, name="wo1")
                nc.sync.dma_start(wo0[:], io["wo"][0:64, :])
                nc.sync.dma_start(wo1[:], io["wo"][64:128, :])
                for st in range(32):
                    g = slice(st * 128, (st + 1) * 128)
                    for db in range(2):
                        dsl = slice(db * 512, (db + 1) * 512)
                        pp = P()
                        nc.tensor.matmul(pp[:], oT0[:, g], wo0[:, dsl],
                                         start=True, stop=False)
                        nc.tensor.matmul(pp[:], oT1[:, g], wo1[:, dsl],
                                         start=False, stop=True)
                        ab = att.tile([128, 512], f32, tag="ab", name="ab")
                        nc.scalar.copy(ab[:], pp[:])
                        nc.sync.dma_start(ar_in[g, dsl], ab[:])

        nc.gpsimd.collective_compute(
            "AllReduce", OP.add, replica_groups=[list(range(NC))],
            ins=[ar_in.opt()], outs=[ar_out.opt()])

        # =================================================================
        # Phase B: MoE
        # =================================================================
        with tc.tile_pool(name="bpool", bufs=1) as bpool, \
             tc.tile_pool(name="hnt", bufs=2) as hnt_pool, \
             tc.tile_pool(name="moe", bufs=2) as moe, \
             tc.tile_pool(name="workb", bufs=2) as work:
            wout_t = bpool.tile([128, KC, D], f32r, name="wout_t")
            nc.sync.dma_start(wout_t[:],
                              io["wout"].rearrange("(hc p) d -> p hc d", p=128))
            w1r = bpool.tile([128, KC, KC, 128], f32r, name="w1r")
            w2r = bpool.tile([128, KC, KC, 128], f32r, name="w2r")
            for hcx in range(KC):
                nc.sync.dma_start(w1r[:, :, hcx, :],
                                  io["w1"][hcx].rearrange("(kc p) m -> p kc m", p=128))
                nc.sync.dma_start(w2r[:, :, hcx, :],
                                  io["w2"][hcx].rearrange("(kc p) m -> p kc m", p=128))
            for b in range(B):
                for sb in range(4):
                    hnT = hnt_pool.tile([128, KC, 512], f32r, tag="hnT", name="hnT")
                    for q4 in range(4):
                        st = b * 16 + sb * 4 + q4
                        def save_h(ht, g=slice(st * 128, (st + 1) * 128)):
                            nc.sync.dma_start(io["h_out"][g, :], ht[:])
                        norm_transpose(work, ar_out, hnT, st, q4 * 128,
                                       extra=save_h, src2=io["x"])
                    # router
                    plog = P((E, 512))
                    for kc in range(KC):
                        nc.tensor.matmul(plog[:], cst["rw"][:, kc, :], hnT[:, kc, :],
                                         start=(kc == 0), stop=(kc == KC - 1))
                    lsb = work.tile([E, 512], f32, tag="lsb", name="lsb")
                    nc.scalar.copy(lsb[:], plog[:])
                    for q4 in range(4):
                        st = b * 16 + sb * 4 + q4
                        g = slice(st * 128, (st + 1) * 128)
                        ptr = P((128, E))
                        nc.tensor.transpose(ptr[:], lsb[:, q4 * 128:(q4 + 1) * 128],
                                            cst["ident"][0:E, 0:E])
                        nt = work.tile([128, E], f32, tag="nt", name="nt")
                        nc.sync.dma_start(nt[:], io["noise"][g, :])
                        zt = work.tile([128, E], f32, tag="zt", name="zt")
                        nc.vector.tensor_tensor(zt[:], ptr[:], nt[:], op=OP.add)
                        ez = work.tile([128, E], f32, tag="ez", name="ez")
                        den = work.tile([128, 1], f32, tag="den", name="den")
                        nc.scalar.activation(ez[:], zt[:], AF.Exp, accum_out=den[:])
                        rd = work.tile([128, 1], f32, tag="den", name="rd")
                        nc.vector.reciprocal(rd[:], den[:])
                        pr = work.tile([128, E], f32, tag="pr", name="pr")
                        nc.vector.tensor_scalar_mul(pr[:], ez[:], rd[:])
                        pet = work.tile([128, E], f32, tag="pet", name="pet")
                        nc.vector.tensor_tensor(pet[:], pr[:], cst["sel"][:],
                                                op=OP.mult)
                        pe = work.tile([128, 1], f32, tag="pe", name="pe")
                        nc.vector.reduce_sum(pe[:], pet[:], axis=AX.X)
                        gtt = work.tile([128, E], f32, tag="gtt", name="gtt")
                        nc.vector.tensor_scalar(gtt[:], pr[:], pe[:], None,
                                                op0=OP.is_gt)
                        cnt = work.tile([128, 1], f32, tag="cnt", name="cnt")
                        nc.vector.reduce_sum(cnt[:], gtt[:], axis=AX.X)
                        ind = work.tile([128, 1], f32, tag="cnt", name="ind")
                        nc.vector.tensor_single_scalar(ind[:], cnt[:], 1.5,
                                                       op=OP.is_lt)
                        nc.vector.tensor_tensor(wgt_all[:, st:st + 1], pe[:], ind[:],
                                                op=OP.mult)
                    # expert FFN
                    ht = moe.tile([128, KC, 512], f32r, tag="ht", name="ht", bufs=1)
                    for hc in range(KC):
                        p1 = P()
                        for kc in range(KC):
                            nc.tensor.matmul(p1[:], w1r[:, kc, hc, :], hnT[:, kc, :],
                                             start=(kc == 0), stop=(kc == KC - 1))
                        p2 = P()
                        for kc in range(KC):
                            nc.tensor.matmul(p2[:], w2r[:, kc, hc, :], hnT[:, kc, :],
                                             start=(kc == 0), stop=(kc == KC - 1))
                        s1 = work.tile([128, 512], f32, tag="s1", name="s1")
                        nc.scalar.activation(s1[:], p1[:], AF.Silu,
                                             bias=cst["b1h"][:, hc:hc + 1], scale=1.0)
                        nc.vector.scalar_tensor_tensor(
                            ht[:, hc, :], p2[:], cst["b2h"][:, hc:hc + 1], s1[:],
                            op0=OP.add, op1=OP.mult)
                    for q4 in range(4):
                        st = b * 16 + sb * 4 + q4
                        g = slice(st * 128, (st + 1) * 128)
                        for db in range(2):
                            peo = P()
                            for hc in range(KC):
                                nc.tensor.matmul(
                                    peo[:], ht[:, hc, q4 * 128:(q4 + 1) * 128],
                                    wout_t[:, hc, db * 512:(db + 1) * 512],
                                    start=(hc == 0), stop=(hc == KC - 1))
                            ob = work.tile([128, 512], f32, tag="ob", name="ob")
                            nc.vector.tensor_scalar_mul(ob[:], peo[:],
                                                        wgt_all[:, st:st + 1])
                            nc.sync.dma_start(io["out"][g, db * 512:(db + 1) * 512],
                                                ob[:])

    nc.compile()
    return nc


# =====================================================================
# Host-side input prep / output combine
# =====================================================================
def prep_in_maps(inputs):
    x = np.asarray(inputs["x"], np.float32).reshape(T, D)
    scale1 = np.asarray(inputs["scale1"], np.float32)
    scale2 = np.asarray(inputs["scale2"], np.float32)
    wq = scale1[:, None] * np.asarray(inputs["wq"], np.float32)
    wk = scale1[:, None] * np.asarray(inputs["wk"], np.float32)
    wv = scale1[:, None] * np.asarray(inputs["wv"], np.float32)
    wo = np.asarray(inputs["wo"], np.float32)
    rw = scale2[:, None] * np.asarray(inputs["router_w"], np.float32)
    w1 = scale2[None, :, None] * np.asarray(inputs["w1"], np.float32)
    w2 = scale2[None, :, None] * np.asarray(inputs["w2"], np.float32)
    wout = np.asarray(inputs["wout"], np.float32)
    b1 = np.asarray(inputs["b1"], np.float32)
    b2 = np.asarray(inputs["b2"], np.float32)

    import jax
    noise = np.asarray(jax.random.gumbel(jax.random.key(42), (B, S, E),
                                         np.float32)) * 0.05
    noise = noise.reshape(T, E).astype(np.float32)

    half = DK // 2
    inv = 1.0 / (10000.0 ** (np.arange(half, dtype=np.float32) / half))
    ang = np.arange(S, dtype=np.float32)[:, None] * inv[None, :]  # [S, 32]
    cos_h = np.cos(ang).T  # [32, S]
    sin_h = np.sin(ang).T
    blk_cos = np.concatenate([cos_h, cos_h], 0)        # [64, S]
    blk_sin = np.concatenate([sin_h, sin_h], 0)
    cosb = np.concatenate([blk_cos, blk_cos], 0).astype(np.float32)  # [128, S]
    sinb = np.concatenate([blk_sin, blk_sin], 0).astype(np.float32)

    masks = np.zeros((128, 4, 512), np.float32)
    kr = np.arange(128)[:, None]
    qc = np.arange(512)[None, :]
    for t in range(4):
        masks[:, t, :] = np.where(kr + 128 * t <= qc, 0.0, MASKNEG)

    ident = np.eye(128, dtype=np.float32)
    id64 = np.zeros((128, 128), np.float32)
    id64[64:128, 0:64] = np.eye(64, dtype=np.float32)

    in_maps = []
    for c in range(NC):
        cols = slice(c * 128, (c + 1) * 128)
        wq_c = np.ascontiguousarray(wq[:, cols])
        wk_c = np.ascontiguousarray(wk[:, cols])
        wv_c = np.ascontiguousarray(wv[:, cols])
        def swap(w):
            ws = np.empty_like(w)
            for hh in range(2):
                r = hh * 64
                ws[:, r:r + 32] = -w[:, r + 32:r + 64]
                ws[:, r + 32:r + 64] = w[:, r:r + 32]
            return ws
        w1_c = np.stack([np.ascontiguousarray(w1[c][:, i * 128:(i + 1) * 128])
                         for i in range(KC)], 0)
        w2_c = np.stack([np.ascontiguousarray(w2[c][:, i * 128:(i + 1) * 128])
                         for i in range(KC)], 0)
        sel = np.zeros((128, E), np.float32)
        sel[:, c] = 1.0
        in_maps.append({
            "x": x, "wq": wq_c, "wk": wk_c, "wv": wv_c,
            "wqs": swap(wq_c), "wks": swap(wk_c),
            "wo": np.ascontiguousarray(wo[cols, :]),
            "cosb": cosb, "sinb": sinb, "masks": masks,
            "ident": ident, "id64": id64,
            "rw": rw, "noise": noise, "sel": sel,
            "w1": w1_c, "w2": w2_c,
            "wout": np.ascontiguousarray(wout[c]),
            "b1h": np.ascontiguousarray(b1[c].reshape(KC, 128).T),
            "b2h": np.ascontiguousarray(b2[c].reshape(KC, 128).T),
        })
    return in_maps


def combine(results):
    y = results[0]["h_out"].astype(np.float64)
    for c in range(NC):
        y = y + results[c]["out"].astype(np.float64)
    return y.astype(np.float32).reshape(B, S, D)


# ---------------------------------------------------------------------
# PJRT runner (axon): persistent jitted executable for the SPMD launch.
# ---------------------------------------------------------------------
import jax
from jax.sharding import Mesh, PartitionSpec
from jax.experimental.shard_map import shard_map
from concourse import bass2jax

import numpy as np
import jax
from jax.sharding import Mesh, PartitionSpec
from jax.experimental.shard_map import shard_map
import concourse.bass as bass
import concourse.mybir as mybir
from concourse import bass2jax


def make_runner(nc, n_cores):
    bass2jax.install_neuronx_cc_hook()
    partition_name = nc.partition_id_tensor.name if nc.partition_id_tensor else None
    in_names, out_names, out_avals, zero_outs = [], [], [], []
    for alloc in nc.m.functions[0].allocations:
        if not isinstance(alloc, mybir.MemoryLocationSet):
            continue
        name = alloc.memorylocations[0].name
        if alloc.kind == "ExternalInput":
            if name != partition_name:
                in_names.append(name)
        elif alloc.kind == "ExternalOutput":
            out_names.append(name)
            shape = tuple(alloc.tensor_shape)
            dtype = mybir.dt.np(alloc.dtype)
            out_avals.append(jax.core.ShapedArray(shape, dtype))
            zero_outs.append(np.zeros(shape, dtype))
    n_params = len(in_names)
    n_outs = len(out_avals)
    all_in_names = list(in_names) + list(out_names)
    if partition_name is not None:
        all_in_names.append(partition_name)

    def _body(*args):
        operands = list(args)
        if partition_name is not None:
            operands.append(bass2jax.partition_id_tensor())
        outs = bass2jax._bass_exec_p.bind(
            *operands,
            out_avals=tuple(out_avals),
            in_names=tuple(all_in_names),
            out_names=tuple(out_names),
            lowering_input_output_aliases=(),
            sim_require_finite=True,
            sim_require_nnan=True,
            nc=nc,
        )
        return tuple(outs)

    devices = jax.devices()[:n_cores]
    mesh = Mesh(np.asarray(devices), ("core",))
    in_specs = (PartitionSpec("core"),) * (n_params + n_outs)
    out_specs = (PartitionSpec("core"),) * n_outs
    donate = tuple(range(n_params, n_params + n_outs))
    sharded = jax.jit(
        shard_map(_body, mesh=mesh, in_specs=in_specs, out_specs=out_specs,
                  check_rep=False),
        donate_argnums=donate, keep_unused=True,
    )

    def run(in_maps):
        per_core = [[np.asarray(m[name]) for name in in_names] for m in in_maps]
        concat_in = [np.concatenate([per_core[c][i] for c in range(n_cores)], axis=0)
                     for i in range(n_params)]
        concat_zeros = [np.zeros((n_cores * z.shape[0], *z.shape[1:]), z.dtype)
                        for z in zero_outs]
        out_arrs = sharded(*concat_in, *concat_zeros)
        out_arrs = [np.asarray(o) for o in out_arrs]
        return [
            {name: out_arrs[i].reshape(n_cores, *out_avals[i].shape)[c]
             for i, name in enumerate(out_names)}
            for c in range(n_cores)
        ]

    return run


_CACHE = {}


def kernel(**inputs):
    if "nc" not in _CACHE:
        _CACHE["nc"] = build_program()
        _CACHE["run"] = make_runner(_CACHE["nc"], NC)
    in_maps = prep_in_maps(inputs)
    results = _CACHE["run"](in_maps)
    return combine(results)



# revision 19
# speedup vs baseline: 1.6526x; 1.6526x over previous
"""Self-contained Trainium2 Bass kernel for nn_DariushLayer_14087492731059.

kernel(**inputs) takes the FULL unsharded inputs of reference.setup_inputs()
and returns the full [B, S, D] float32 output, computed across 8 NeuronCores.

Parallelization:
  - Attention tensor-parallel over heads (2 heads/core); out-projection
    partials are ReduceScattered in bf16 as 4 quarter-batch collectives that
    hide under attention compute.
  - The RS outputs are AllGathered immediately (8 chunked bf16 AllGathers,
    512 tokens each, zero compute between RS and AG) and each core does
    the +x residual, rmsnorm and transpose per gathered chunk locally,
    pipelined against the expert FFN (expert-parallel, 1 expert/core).
  - GEMMs: fp32r stationary for QKV, bf16 elsewhere (same matmul rate as
    fp32r in these regimes; bf16 halves SBUF/DMA/collective bytes).
"""

import numpy as np
import concourse.bass as bass
import concourse.tile as tile
from concourse import bacc, mybir
from contextlib import ExitStack

f32, f32r, bf16 = mybir.dt.float32, mybir.dt.float32r, mybir.dt.bfloat16
AF = mybir.ActivationFunctionType
OP = mybir.AluOpType
AX = mybir.AxisListType

B, S, D, H, DK, E = 2, 2048, 1024, 16, 64, 8
T = B * S
NC = 8
KC = D // 128
NCH = 8             # AllGather chunks (64 tokens per core per chunk)
EPS = 1e-6
MASKNEG = -30000.0
# stream_shuffle mask: swap 16-halves within each 32-partition quadrant
SHUF = list(range(16, 32)) + list(range(16))


def build_program():
    nc = bacc.Bacc("TRN2", target_bir_lowering=False, debug=False, num_devices=NC)
    dt = nc.dram_tensor
    io = {}
    def inp(nm, shp, ty=f32):
        io[nm] = dt(nm, shp, ty, kind="ExternalInput").ap()
    def outp(nm, shp, ty=f32):
        io[nm] = dt(nm, shp, ty, kind="ExternalOutput").ap()
    inp("x", [T, D])
    inp("xb", [T, D], bf16)
    for nm in ("wq", "wk", "wv"):
        inp(nm, [D, 128], bf16)
    inp("wo0", [64, D], bf16)
    inp("wo1", [64, D], bf16)
    inp("cosb", [128, S]); inp("sinb", [128, S])
    inp("masks", [128, 4, 512])
    inp("identb", [128, 128], bf16); inp("id64b", [128, 128], bf16)
    inp("identf", [128, 128], f32)
    inp("rw", [128, KC, E], bf16)
    inp("noise", [T, E])
    inp("sel", [128, E])
    inp("w1h", [128, KC, KC, 128], bf16)
    inp("w2h", [128, KC, KC, 128], bf16)
    inp("wouth", [128, KC, D], bf16)
    inp("b1h", [128, KC]); inp("b2h", [128, KC])
    outp("out", [T, D])
    outp("h_out", [512, D], bf16)      # raw attention sums for owned shards

    noise_v = io["noise"].rearrange("(b hf c q t) e -> b hf q c t e",
                                    b=2, hf=2, c=8, q=2, t=64)
    x_v = io["xb"].rearrange("(b hf c q t) d -> b hf q c t d",
                             b=2, hf=2, c=8, q=2, t=64)

    with tile.TileContext(nc) as tc, ExitStack() as top:
        const = top.enter_context(tc.tile_pool(name="const", bufs=1))
        psum = top.enter_context(tc.tile_pool(name="psum", bufs=1, space="PSUM"))
        dram = top.enter_context(tc.tile_pool(name="dram", bufs=1, space="DRAM"))
        moew = top.enter_context(tc.tile_pool(name="moew", bufs=1))
        moe = top.enter_context(tc.tile_pool(name="moe", bufs=2))
        fwork = top.enter_context(tc.tile_pool(name="fwork", bufs=2))

        def P(shape=(128, 512), ty=f32, tag="ps", bufs=4):
            return psum.tile(list(shape), ty, tag=tag, name=tag, bufs=bufs)

        cst = {}
        # identb is needed within ~5us (first transposes) - load it first
        cst["identb"] = const.tile([128, 128], bf16, name="identb")
        nc.sync.dma_start(cst["identb"][:], io["identb"][:])
        for nm, shp, ty in [("id64b", [128, 128], bf16),
                            ("identf", [128, 128], f32), ("sel", [128, E], f32),
                            ("b1h", [128, KC], f32), ("b2h", [128, KC], f32),
                            ("rw", [128, KC, E], bf16)]:
            cst[nm] = const.tile(shp, ty, name=nm)
        def load_late_consts():
            for nm in ("id64b", "identf", "sel", "b1h", "b2h", "rw"):
                nc.sync.dma_start(cst[nm][:], io[nm][:])
        vtail = const.tile([128, 2], bf16, name="vtail")
        nc.vector.memset(vtail[:, 0:1], 1.0)
        nc.vector.memset(vtail[:, 1:2], 0.0)
        sqscr = const.tile([128, D], f32, name="sqscr")
        eps_t = const.tile([128, 1], f32, name="eps_t")
        nc.vector.memset(eps_t[:], EPS)

        # DRAM scratch
        arq = [dram.tile([1024, D], bf16, name=f"arq{r}") for r in range(4)]
        rsq = [dram.tile([128, D], bf16, name=f"rsq{r}") for r in range(4)]
        agout = [dram.tile([512, D], bf16, name=f"agout{j}",
                           addr_space="Shared") for j in range(NCH)]

        # persistent FFN weights
        w1s = moew.tile([128, KC, KC, 128], bf16, name="w1s")
        w2s = moew.tile([128, KC, KC, 128], bf16, name="w2s")
        wos = moew.tile([128, KC, D], bf16, name="wos")

        GRP = [list(range(NC))]

        MARKS = []
        def mark(label):
            MARKS.append((label, nc.next_id()))
        nc._marks = MARKS

        # ---------------- helpers ----------------
        def rmsnorm(work, xt, out_bf, tagpfx=""):
            """out_bf = xt * rsqrt(mean(xt^2) + EPS), bf16."""
            ssum = work.tile([128, 1], f32, tag=tagpfx + "ss", name="ssum", bufs=4)
            nc.scalar.activation(sqscr[:], xt[:], AF.Square, accum_out=ssum[:])
            sd = work.tile([128, 1], f32, tag=tagpfx + "ss", name="sd", bufs=4)
            nc.scalar.activation(sd[:], ssum[:], AF.Sqrt, bias=eps_t[:],
                                 scale=1.0 / D)
            rr = work.tile([128, 1], f32, tag=tagpfx + "ss", name="rr", bufs=4)
            nc.vector.reciprocal(rr[:], sd[:])
            nc.vector.tensor_scalar_mul(out_bf[:], xt[:], rr[:])

        def ffn_chunk(j):
            bj, hfj, qj = j // 4, (j // 2) % 2, j % 2
            # assemble normed+transposed hn for this chunk's 512 tokens
            hnT = moe.tile([128, KC, 512], bf16, tag="hnT", name="hnT")
            for q4 in range(4):
                agrow = fwork.tile([128, D], bf16, tag="agrow", name="agrow")
                nc.scalar.dma_start(agrow[:],
                                    agout[j][q4 * 128:(q4 + 1) * 128, :])
                xrow = fwork.tile([128, D], bf16, tag="xrow", name="xrow")
                nc.sync.dma_start(xrow[:],
                                  x_v[bj, hfj, qj, 2 * q4:2 * q4 + 2, :, :])
                hxr = fwork.tile([128, D], f32, tag="hxr", name="hxr")
                nc.vector.tensor_tensor(hxr[:], agrow[:], xrow[:], op=OP.add)
                hhx = fwork.tile([128, D], bf16, tag="hhx", name="hhx")
                rmsnorm(fwork, hxr, hhx, tagpfx="f")
                for kc in range(KC):
                    pt = P((128, 128), bf16, tag="pt", bufs=2)
                    nc.tensor.transpose(pt[:], hhx[:, kc * 128:(kc + 1) * 128],
                                        cst["identb"][:])
                    if kc % 2 == 0:
                        nc.scalar.copy(hnT[:, kc, q4 * 128:(q4 + 1) * 128],
                                       pt[:])
                    else:
                        nc.vector.tensor_copy(
                            hnT[:, kc, q4 * 128:(q4 + 1) * 128], pt[:])
            # router on this chunk's 512 tokens
            plog = P((E, 512), f32, tag="ps")
            for kc in range(KC):
                nc.tensor.matmul(plog[:], cst["rw"][:, kc, :], hnT[:, kc, :],
                                 start=(kc == 0), stop=(kc == KC - 1))
            lsb = fwork.tile([E, 512], f32, tag="lsb", name="lsb", bufs=1)
            nc.scalar.copy(lsb[:], plog[:])
            wgt = fwork.tile([128, 4], f32, tag="wgt", name="wgt")
            for q4 in range(4):
                ptr = P((128, E), f32, tag="pt", bufs=2)
                nc.tensor.transpose(ptr[:], lsb[:, q4 * 128:(q4 + 1) * 128],
                                    cst["identf"][0:E, 0:E])
                nt = fwork.tile([128, E], f32, tag="nt", name="nt")
                nc.sync.dma_start(nt[:],
                                  noise_v[bj, hfj, qj, 2 * q4:2 * q4 + 2, :, :])
                zt = fwork.tile([128, E], f32, tag="zt", name="zt")
                nc.vector.tensor_tensor(zt[:], ptr[:], nt[:], op=OP.add)
                ez = fwork.tile([128, E], f32, tag="ez", name="ez")
                den = fwork.tile([128, 1], f32, tag="den", name="den")
                nc.scalar.activation(ez[:], zt[:], AF.Exp, accum_out=den[:])
                rd = fwork.tile([128, 1], f32, tag="rd", name="rd")
                nc.vector.reciprocal(rd[:], den[:])
                pet = fwork.tile([128, E], f32, tag="pet", name="pet")
                nc.vector.tensor_tensor(pet[:], ez[:], cst["sel"][:], op=OP.mult)
                peu = fwork.tile([128, 1], f32, tag="peu", name="peu")
                nc.vector.reduce_sum(peu[:], pet[:], axis=AX.X)
                gtt = fwork.tile([128, E], f32, tag="gtt", name="gtt")
                nc.vector.tensor_scalar(gtt[:], ez[:], peu[:], None, op0=OP.is_gt)
                cnt = fwork.tile([128, 1], f32, tag="cnt", name="cnt")
                nc.vector.reduce_sum(cnt[:], gtt[:], axis=AX.X)
                ind = fwork.tile([128, 1], f32, tag="ind", name="ind")
                nc.vector.tensor_single_scalar(ind[:], cnt[:], 1.5, op=OP.is_lt)
                pw = fwork.tile([128, 1], f32, tag="pw", name="pw")
                nc.vector.tensor_tensor(pw[:], peu[:], rd[:], op=OP.mult)
                nc.vector.tensor_tensor(wgt[:, q4:q4 + 1], pw[:], ind[:],
                                        op=OP.mult)
            # expert SwiGLU
            ht = moe.tile([128, KC, 512], bf16, tag="ht", name="ht", bufs=1)
            for hc in range(KC):
                p1 = P(tag="ps")
                for kc in range(KC):
                    nc.tensor.matmul(p1[:], w1s[:, kc, hc, :], hnT[:, kc, :],
                                     start=(kc == 0), stop=(kc == KC - 1))
                p2 = P(tag="ps")
                for kc in range(KC):
                    nc.tensor.matmul(p2[:], w2s[:, kc, hc, :], hnT[:, kc, :],
                                     start=(kc == 0), stop=(kc == KC - 1))
                s1 = fwork.tile([128, 512], f32, tag="s1", name="s1")
                nc.scalar.activation(s1[:], p1[:], AF.Silu,
                                     bias=cst["b1h"][:, hc:hc + 1], scale=1.0)
                nc.vector.scalar_tensor_tensor(
                    ht[:, hc, :], p2[:], cst["b2h"][:, hc:hc + 1], s1[:],
                    op0=OP.add, op1=OP.mult)
            # out-projection + gate + scatter to global rows
            for q4 in range(4):
                ob = fwork.tile([128, D], f32, tag="ob", name="ob")
                for db in range(2):
                    dsl = slice(db * 512, (db + 1) * 512)
                    peo = P(tag="ps")
                    for hc in range(KC):
                        nc.tensor.matmul(
                            peo[:], ht[:, hc, q4 * 128:(q4 + 1) * 128],
                            wos[:, hc, dsl],
                            start=(hc == 0), stop=(hc == KC - 1))
                    nc.vector.tensor_scalar_mul(ob[:, dsl], peo[:],
                                                wgt[:, q4:q4 + 1])
                gb0 = bj * 2048 + hfj * 1024 + (2 * q4) * 128 + qj * 64
                gb1 = gb0 + 128
                nc.sync.dma_start(io["out"][gb0:gb0 + 64, :], ob[0:64, :])
                nc.sync.dma_start(io["out"][gb1:gb1 + 64, :], ob[64:128, :])

        # =================================================================
        # Phase A: attention (TP over heads) with streamed RS -> AG
        # =================================================================
        with tc.tile_pool(name="apool", bufs=1) as apool, \
             tc.tile_pool(name="awork", bufs=2) as work, \
             tc.tile_pool(name="xnt", bufs=1) as xnt_pool, \
             tc.tile_pool(name="vsp", bufs=16) as vsp, \
             tc.tile_pool(name="expp", bufs=2) as expp:
            cst["masks"] = apool.tile([128, 4, 512], f32, name="masks")
            for nm in ("wq", "wk", "wv"):
                cst[nm] = apool.tile([128, KC, 128], bf16, name=nm)
            wo_t = [apool.tile([64, D], bf16, name=f"wo{h}") for h in range(2)]
            def load_qkv_consts():
                for nm in ("wq", "wk", "wv"):
                    nc.sync.dma_start(
                        cst[nm][:], io[nm].rearrange("(kc p) m -> p kc m", p=128))
            def load_attn_consts():
                nc.sync.dma_start(cst["masks"][:], io["masks"][:])
                nc.sync.dma_start(wo_t[0][:], io["wo0"][:])
                nc.sync.dma_start(wo_t[1][:], io["wo1"][:])
            qb = apool.tile([128, S], bf16, name="qb")
            kb = apool.tile([128, S], bf16, name="kb")
            vb = apool.tile([128, S], bf16, name="vb")
            oT = [apool.tile([66, S], bf16, name=f"oT{h}") for h in range(2)]
            rcp = [apool.tile([128, 16], f32, name=f"rcp{h}") for h in range(2)]
            vs_all = {}

            def qkv_block(b, sb):
                xnT = xnt_pool.tile([128, KC, 512], bf16, tag="xnT", name="xnT")
                for q4 in range(4):
                    st = b * 16 + sb * 4 + q4
                    xt = work.tile([128, D], f32, tag="xt", name="xt")
                    nc.sync.dma_start(xt[:], io["x"][st * 128:(st + 1) * 128, :])
                    xh = work.tile([128, D], bf16, tag="xh", name="xh")
                    rmsnorm(work, xt, xh)
                    for kc in range(KC):
                        pt = P((128, 128), bf16, tag="pt", bufs=2)
                        nc.tensor.transpose(pt[:], xh[:, kc * 128:(kc + 1) * 128],
                                            cst["identb"][:])
                        if kc % 2 == 0:
                            nc.scalar.copy(xnT[:, kc, q4 * 128:(q4 + 1) * 128],
                                           pt[:])
                        else:
                            nc.vector.tensor_copy(
                                xnT[:, kc, q4 * 128:(q4 + 1) * 128], pt[:])
                sl = slice(sb * 512, (sb + 1) * 512)
                ct = work.tile([128, 512], f32, tag="cos", name="ct")
                nc.sync.dma_start(ct[:], io["cosb"][:, sl])
                snt = work.tile([128, 512], f32, tag="sin", name="snt")
                nc.sync.dma_start(snt[:], io["sinb"][:, sl])
                for wname, dstT in (("wq", qb), ("wk", kb)):
                    pa = P(tag="ps")
                    for kc in range(KC):
                        nc.tensor.matmul(pa[:], cst[wname][:, kc, :],
                                         xnT[:, kc, :],
                                         start=(kc == 0), stop=(kc == KC - 1))
                    qs = work.tile([128, 512], f32, tag="qs", name="qs", bufs=1)
                    nc.vector.stream_shuffle(qs[:], pa[:], SHUF)
                    t1 = work.tile([128, 512], bf16, tag="t1", name="t1", bufs=1)
                    nc.vector.tensor_tensor(t1[:], pa[:], ct[:], op=OP.mult)
                    t2 = work.tile([128, 512], bf16, tag="t2", name="t2", bufs=1)
                    nc.gpsimd.tensor_tensor(t2[:], qs[:], snt[:], op=OP.mult)
                    nc.gpsimd.tensor_tensor(dstT[:, sl], t1[:], t2[:], op=OP.add)
                pv = P(tag="ps")
                for kc in range(KC):
                    nc.tensor.matmul(pv[:], cst["wv"][:, kc, :], xnT[:, kc, :],
                                     start=(kc == 0), stop=(kc == KC - 1))
                nc.scalar.copy(vb[:, sl], pv[:])

            def att_half(b, half):
                for h in range(2):
                    hr = slice(h * 64, (h + 1) * 64)
                    idn = (cst["identb"][0:64, 0:64] if h == 0
                           else cst["id64b"][64:128, 0:64])
                    for m in range(half * 8, half * 8 + 8):
                        gk = slice(m * 128, (m + 1) * 128)
                        ptv = P((128, 64), bf16, tag="pt", bufs=2)
                        nc.tensor.transpose(ptv[:], vb[hr, gk], idn)
                        vs = vsp.tile([128, 66], bf16, tag=f"vs{h}", name="vs")
                        if m % 2 == 0:
                            nc.scalar.copy(vs[:, 0:64], ptv[:])
                        else:
                            nc.vector.tensor_copy(vs[:, 0:64], ptv[:])
                        nc.vector.tensor_copy(vs[:, 64:66], vtail[:])
                        vs_all[(h, m)] = vs
                    for jq in (2 * half, 2 * half + 1):
                        gq = slice(jq * 512, (jq + 1) * 512)
                        pos = P(tag="pos", bufs=2)
                        nch = 4 * jq + 4
                        for m in range(nch):
                            gk = slice(m * 128, (m + 1) * 128)
                            pse = P(tag="ps")
                            nc.tensor.matmul(pse[:], kb[hr, gk], qb[hr, gq],
                                             start=True, stop=True)
                            if m >= 4 * jq:
                                nc.vector.tensor_tensor(
                                    pse[:], pse[:],
                                    cst["masks"][:, m - 4 * jq, :], op=OP.add)
                            et = expp.tile([128, 512], bf16, tag="et", name="et")
                            nc.scalar.activation(et[:], pse[:], AF.Exp,
                                                 scale=0.125)
                            nc.tensor.matmul(pos[0:66, :], vs_all[(h, m)][:],
                                             et[:], start=(m == 0),
                                             stop=(m == nch - 1))
                        nc.scalar.copy(oT[h][:, gq], pos[0:66, :])
                        for qt in range(4):
                            st16 = jq * 4 + qt
                            g1 = slice(jq * 512 + qt * 128,
                                       jq * 512 + (qt + 1) * 128)
                            pd = P((128, 1), bf16, tag="pt", bufs=2)
                            nc.tensor.transpose(pd[:], oT[h][64:65, g1],
                                                cst["identb"][64:65, 64:65])
                            nc.vector.reciprocal(rcp[h][:, st16:st16 + 1], pd[:])

            def outproj_half(b, half):
                r = b * 2 + half
                for k16 in range(8):
                    st16 = half * 8 + k16
                    g = slice(st16 * 128, (st16 + 1) * 128)
                    for db in range(2):
                        dsl = slice(db * 512, (db + 1) * 512)
                        pp0 = P(tag="ps")
                        nc.tensor.matmul(pp0[:], oT[0][0:64, g], wo_t[0][:, dsl],
                                         start=True, stop=True)
                        pp1 = P(tag="ps")
                        nc.tensor.matmul(pp1[:], oT[1][0:64, g], wo_t[1][:, dsl],
                                         start=True, stop=True)
                        abt = work.tile([128, 512], f32, tag="ab", name="abt",
                                        bufs=1)
                        nc.scalar.activation(abt[:], pp0[:], AF.Copy,
                                             scale=rcp[0][:, st16:st16 + 1])
                        ab2 = work.tile([128, 512], bf16, tag="ab2", name="ab2")
                        nc.vector.scalar_tensor_tensor(
                            ab2[:], pp1[:], rcp[1][:, st16:st16 + 1], abt[:],
                            op0=OP.mult, op1=OP.add)
                        nc.gpsimd.dma_start(
                            arq[r][k16 * 128:(k16 + 1) * 128, dsl], ab2[:])
                nc.gpsimd.collective_compute(
                    "ReduceScatter", OP.add, replica_groups=GRP,
                    ins=[arq[r][:]], outs=[rsq[r][:]])
                # shard -> h_out + immediate AllGathers (no compute between)
                nc.gpsimd.dma_start(io["h_out"][r * 128:(r + 1) * 128, :],
                                    rsq[r][:])
                for qj in range(2):
                    nc.gpsimd.collective_compute(
                        "AllGather", OP.bypass, replica_groups=GRP,
                        ins=[rsq[r][qj * 64:(qj + 1) * 64, :]],
                        outs=[agout[2 * r + qj][:]])

            # ---- batch 0 ----
            mark("qkv_b0")
            load_qkv_consts()
            qkv_block(0, 0)
            load_attn_consts()
            load_late_consts()
            qkv_block(0, 1)
            qkv_block(0, 2)
            qkv_block(0, 3)
            mark("att(0,0)")
            att_half(0, 0)
            mark("outproj(0,0)")
            outproj_half(0, 0)        # RS_0, AG_0, AG_1
            mark("att(0,1)")
            att_half(0, 1)
            mark("outproj(0,1)")
            outproj_half(0, 1)        # RS_1, AG_2, AG_3
            # FFN weights: transfer overlaps batch-1 attention
            nc.sync.dma_start(w1s[:], io["w1h"][:])
            nc.sync.dma_start(w2s[:], io["w2h"][:])
            nc.sync.dma_start(wos[:], io["wouth"][:])
            # ---- batch 1 ----
            mark("qkv_b1")
            for sb in range(4):
                qkv_block(1, sb)
            mark("att(1,0)")
            att_half(1, 0)
            mark("ffn0")
            ffn_chunk(0)
            mark("outproj(1,0)")
            outproj_half(1, 0)        # RS_2, AG_4, AG_5
            mark("att(1,1)")
            att_half(1, 1)
            mark("ffn1")
            ffn_chunk(1)
            mark("outproj(1,1)")
            outproj_half(1, 1)        # RS_3, AG_6, AG_7

        # =================================================================
        # Phase B: remaining expert FFN chunks
        # =================================================================
        for j in range(2, NCH):
            mark(f"ffn{j}")
            ffn_chunk(j)

    nc.compile()
    return nc


# =====================================================================
# Host-side input prep / output combine
# =====================================================================
def _bf16(a):
    import ml_dtypes
    return np.asarray(a, np.float32).astype(ml_dtypes.bfloat16)


def prep_in_maps(inputs):
    x = np.asarray(inputs["x"], np.float32).reshape(T, D)
    scale1 = np.asarray(inputs["scale1"], np.float32)
    scale2 = np.asarray(inputs["scale2"], np.float32)
    wq = scale1[:, None] * np.asarray(inputs["wq"], np.float32)
    wk = scale1[:, None] * np.asarray(inputs["wk"], np.float32)
    wv = scale1[:, None] * np.asarray(inputs["wv"], np.float32)
    wo = np.asarray(inputs["wo"], np.float32)
    rw = scale2[:, None] * np.asarray(inputs["router_w"], np.float32)
    w1 = scale2[None, :, None] * np.asarray(inputs["w1"], np.float32)
    w2 = scale2[None, :, None] * np.asarray(inputs["w2"], np.float32)
    wout = np.asarray(inputs["wout"], np.float32)
    b1 = np.asarray(inputs["b1"], np.float32)
    b2 = np.asarray(inputs["b2"], np.float32)

    import jax
    noise = np.asarray(jax.random.gumbel(jax.random.key(42), (B, S, E),
                                         np.float32)) * 0.05
    noise = noise.reshape(T, E).astype(np.float32)

    # RoPE tables in the pair-interleaved layout:
    #   col r = hh*64 + 32*q2 + i      (i < 16)  -> x1 dim d = 16*q2 + i
    #   col r = hh*64 + 32*q2 + 16 + i           -> x2 dim d = 16*q2 + i
    half = DK // 2
    inv = 1.0 / (10000.0 ** (np.arange(half, dtype=np.float32) / half))
    ang = np.arange(S, dtype=np.float32)[:, None] * inv[None, :]  # [S, 32]
    cos_h, sin_h = np.cos(ang).T, np.sin(ang).T                   # [32, S]
    perm = np.zeros(128, np.int64)
    cosb = np.zeros((128, S), np.float32)
    sinb = np.zeros((128, S), np.float32)
    for hh in range(2):
        for q2 in range(2):
            for i in range(16):
                d = 16 * q2 + i
                r1 = hh * 64 + 32 * q2 + i
                r2 = r1 + 16
                perm[r1] = hh * 64 + d
                perm[r2] = hh * 64 + 32 + d
                cosb[r1] = cos_h[d]; cosb[r2] = cos_h[d]
                sinb[r1] = -sin_h[d]; sinb[r2] = sin_h[d]

    masks = np.zeros((128, 4, 512), np.float32)
    kr = np.arange(128)[:, None]
    qc = np.arange(512)[None, :]
    for t in range(4):
        masks[:, t, :] = np.where(kr + 128 * t <= qc, 0.0, MASKNEG)

    identb = np.eye(128, dtype=np.float32)
    id64b = np.zeros((128, 128), np.float32)
    id64b[64:128, 0:64] = np.eye(64, dtype=np.float32)

    xb = _bf16(x)
    in_maps = []
    for c in range(NC):
        cols = slice(c * 128, (c + 1) * 128)
        wq_c = _bf16(np.ascontiguousarray(wq[:, cols])[:, perm])
        wk_c = _bf16(np.ascontiguousarray(wk[:, cols])[:, perm])
        wv_c = _bf16(np.ascontiguousarray(wv[:, cols]))
        wo_c = np.ascontiguousarray(wo[cols, :])
        sel = np.zeros((128, E), np.float32)
        sel[:, c] = 1.0
        w1c = w1[c].reshape(KC, 128, KC, 128).transpose(1, 0, 2, 3)
        w2c = w2[c].reshape(KC, 128, KC, 128).transpose(1, 0, 2, 3)
        woc = wout[c].reshape(KC, 128, D).transpose(1, 0, 2)
        rwh = rw.reshape(KC, 128, E).transpose(1, 0, 2)
        in_maps.append({
            "x": x, "xb": xb,
            "wq": wq_c, "wk": wk_c, "wv": wv_c,
            "wo0": _bf16(wo_c[0:64]), "wo1": _bf16(wo_c[64:128]),
            "cosb": cosb, "sinb": sinb, "masks": masks,
            "identb": _bf16(identb), "id64b": _bf16(id64b), "identf": identb,
            "rw": _bf16(rwh), "noise": noise, "sel": sel,
            "w1h": _bf16(w1c), "w2h": _bf16(w2c), "wouth": _bf16(woc),
            "b1h": np.ascontiguousarray(b1[c].reshape(KC, 128).T),
            "b2h": np.ascontiguousarray(b2[c].reshape(KC, 128).T),
        })
    return in_maps


def combine(results):
    x = _COMB_X[0]
    h = np.array(x, np.float64)     # h = x + gathered attention sums
    for c in range(NC):
        hs = np.asarray(results[c]["h_out"], np.float32).astype(np.float64)
        for r in range(4):
            b, hf = r // 2, r % 2
            g0 = b * 2048 + hf * 1024 + c * 128
            h[g0:g0 + 128] += hs[r * 128:(r + 1) * 128]
    y = h
    for c in range(NC):
        y = y + results[c]["out"].astype(np.float64)
    return y.astype(np.float32).reshape(B, S, D)


_COMB_X = [None]


# ---------------------------------------------------------------------
# PJRT runner (axon): persistent jitted executable for the SPMD launch.
# ---------------------------------------------------------------------
import jax
from jax.sharding import Mesh, PartitionSpec
from jax.experimental.shard_map import shard_map
import concourse.mybir as mybir_mod
from concourse import bass2jax


def make_runner(nc, n_cores):
    bass2jax.install_neuronx_cc_hook()
    partition_name = nc.partition_id_tensor.name if nc.partition_id_tensor else None
    in_names, out_names, out_avals, zero_outs = [], [], [], []
    for alloc in nc.m.functions[0].allocations:
        if not isinstance(alloc, mybir_mod.MemoryLocationSet):
            continue
        name = alloc.memorylocations[0].name
        if alloc.kind == "ExternalInput":
            if name != partition_name:
                in_names.append(name)
        elif alloc.kind == "ExternalOutput":
            out_names.append(name)
            shape = tuple(alloc.tensor_shape)
            dtype = mybir_mod.dt.np(alloc.dtype)
            out_avals.append(jax.core.ShapedArray(shape, dtype))
            zero_outs.append(np.zeros(shape, dtype))
    n_params = len(in_names)
    n_outs = len(out_avals)
    all_in_names = list(in_names) + list(out_names)
    if partition_name is not None:
        all_in_names.append(partition_name)

    def _body(*args):
        operands = list(args)
        if partition_name is not None:
            operands.append(bass2jax.partition_id_tensor())
        outs = bass2jax._bass_exec_p.bind(
            *operands,
            out_avals=tuple(out_avals),
            in_names=tuple(all_in_names),
            out_names=tuple(out_names),
            lowering_input_output_aliases=(),
            sim_require_finite=True,
            sim_require_nnan=True,
            nc=nc,
        )
        return tuple(outs)

    devices = jax.devices()[:n_cores]
    mesh = Mesh(np.asarray(devices), ("core",))
    in_specs = (PartitionSpec("core"),) * (n_params + n_outs)
    out_specs = (PartitionSpec("core"),) * n_outs
    donate = tuple(range(n_params, n_params + n_outs))
    sharded = jax.jit(
        shard_map(_body, mesh=mesh, in_specs=in_specs, out_specs=out_specs,
                  check_rep=False),
        donate_argnums=donate, keep_unused=True,
    )

    def run(in_maps):
        per_core = [[np.asarray(m[name]) for name in in_names] for m in in_maps]
        concat_in = [np.concatenate([per_core[c][i] for c in range(n_cores)], axis=0)
                     for i in range(n_params)]
        concat_zeros = [np.zeros((n_cores * z.shape[0], *z.shape[1:]), z.dtype)
                        for z in zero_outs]
        out_arrs = sharded(*concat_in, *concat_zeros)
        out_arrs = [np.asarray(o) for o in out_arrs]
        return [
            {name: out_arrs[i].reshape(n_cores, *out_avals[i].shape)[c]
             for i, name in enumerate(out_names)}
            for c in range(n_cores)
        ]

    return run


_CACHE = {}


def kernel(**inputs):
    if "nc" not in _CACHE:
        _CACHE["nc"] = build_program()
        _CACHE["run"] = make_runner(_CACHE["nc"], NC)
    in_maps = prep_in_maps(inputs)
    _COMB_X[0] = np.asarray(inputs["x"], np.float32).reshape(T, D)
    results = _CACHE["run"](in_maps)
    return combine(results)


# revision 21
# speedup vs baseline: 1.9949x; 1.2071x over previous
"""Self-contained Trainium2 Bass kernel for nn_DariushLayer_14087492731059.

kernel(**inputs) takes the FULL unsharded inputs of reference.setup_inputs()
and returns the full [B, S, D] float32 output, computed across 8 NeuronCores.

Parallelization:
  - Attention tensor-parallel over heads (2 heads/core); out-projection
    partials are ReduceScattered in bf16 as 4 quarter-batch collectives that
    hide under attention compute.
  - The RS outputs are AllGathered immediately (8 chunked bf16 AllGathers,
    512 tokens each, zero compute between RS and AG) and each core does
    the +x residual, rmsnorm and transpose per gathered chunk locally,
    pipelined against the expert FFN (expert-parallel, 1 expert/core).
  - GEMMs: fp32r stationary for QKV, bf16 elsewhere (same matmul rate as
    fp32r in these regimes; bf16 halves SBUF/DMA/collective bytes).
"""

import numpy as np
import concourse.bass as bass
import concourse.tile as tile
from concourse import bacc, mybir
from contextlib import ExitStack

f32, f32r, bf16 = mybir.dt.float32, mybir.dt.float32r, mybir.dt.bfloat16
f8 = mybir.dt.float8e4
DR = mybir.MatmulPerfMode.DoubleRow
AF = mybir.ActivationFunctionType
OP = mybir.AluOpType
AX = mybir.AxisListType

B, S, D, H, DK, E = 2, 2048, 1024, 16, 64, 8
T = B * S
NC = 8
KC = D // 128
NCH = 8             # AllGather chunks (64 tokens per core per chunk)
EPS = 1e-6
MASKNEG = -30000.0
# stream_shuffle mask: swap 16-halves within each 32-partition quadrant
SHUF = list(range(16, 32)) + list(range(16))


def build_program():
    nc = bacc.Bacc("TRN2", target_bir_lowering=False, debug=False, num_devices=NC)
    dt = nc.dram_tensor
    io = {}
    def inp(nm, shp, ty=f32):
        io[nm] = dt(nm, shp, ty, kind="ExternalInput").ap()
    def outp(nm, shp, ty=f32):
        io[nm] = dt(nm, shp, ty, kind="ExternalOutput").ap()
    inp("x", [T, D])
    inp("xb", [T, D], bf16)
    for nm in ("wq", "wk", "wv"):
        inp(nm, [D, 128], bf16)
    inp("wo0", [64, D], bf16)
    inp("wo1", [64, D], bf16)
    inp("cosb", [128, S]); inp("sinb", [128, S])
    inp("masks", [128, 4, 512])
    inp("identb", [128, 128], bf16); inp("id64b", [128, 128], bf16)
    inp("identf", [128, 128], f32)
    inp("rw", [128, KC // 2, 2, E], f8)
    inp("noise", [T, E])
    inp("sel", [128, E])
    inp("w1h", [128, KC // 2, 2, KC, 128], f8)
    inp("w2h", [128, KC // 2, 2, KC, 128], f8)
    inp("wouth", [128, KC // 2, 2, D], f8)
    inp("b1h", [128, KC]); inp("b2h", [128, KC])
    outp("out", [T, D])
    outp("h_out", [512, D], bf16)      # raw attention sums for owned shards

    noise_v = io["noise"].rearrange("(b hf c q t) e -> b hf q c t e",
                                    b=2, hf=2, c=8, q=2, t=64)
    x_v = io["xb"].rearrange("(b hf c q t) d -> b hf q c t d",
                             b=2, hf=2, c=8, q=2, t=64)

    with tile.TileContext(nc) as tc, ExitStack() as top:
        const = top.enter_context(tc.tile_pool(name="const", bufs=1))
        psum = top.enter_context(tc.tile_pool(name="psum", bufs=1, space="PSUM"))
        dram = top.enter_context(tc.tile_pool(name="dram", bufs=1, space="DRAM"))
        moew = top.enter_context(tc.tile_pool(name="moew", bufs=1))
        moe = top.enter_context(tc.tile_pool(name="moe", bufs=2))
        fwork = top.enter_context(tc.tile_pool(name="fwork", bufs=2))

        def P(shape=(128, 512), ty=f32, tag="ps", bufs=4):
            return psum.tile(list(shape), ty, tag=tag, name=tag, bufs=bufs)

        cst = {}
        # identb is needed within ~5us (first transposes) - load it first
        cst["identb"] = const.tile([128, 128], bf16, name="identb")
        nc.sync.dma_start(cst["identb"][:], io["identb"][:])
        for nm, shp, ty in [("id64b", [128, 128], bf16),
                            ("identf", [128, 128], f32), ("sel", [128, E], f32),
                            ("b1h", [128, KC], f32), ("b2h", [128, KC], f32),
                            ("rw", [128, KC // 2, 2, E], f8)]:
            cst[nm] = const.tile(shp, ty, name=nm)
        def load_late_consts():
            for nm in ("id64b", "identf", "sel", "b1h", "b2h", "rw"):
                nc.sync.dma_start(cst[nm][:], io[nm][:])
        vtail = const.tile([128, 2], bf16, name="vtail")
        nc.vector.memset(vtail[:, 0:1], 1.0)
        nc.vector.memset(vtail[:, 1:2], 0.0)
        sqscr = const.tile([128, D], f32, name="sqscr")
        eps_t = const.tile([128, 1], f32, name="eps_t")
        nc.vector.memset(eps_t[:], EPS)

        # DRAM scratch
        arq = [dram.tile([1024, D], bf16, name=f"arq{r}") for r in range(4)]
        rsq = [dram.tile([128, D], bf16, name=f"rsq{r}") for r in range(4)]
        agout = [dram.tile([512, D], bf16, name=f"agout{j}",
                           addr_space="Shared") for j in range(NCH)]

        # persistent FFN weights
        w1s = moew.tile([128, KC // 2, 2, KC, 128], f8, name="w1s")
        w2s = moew.tile([128, KC // 2, 2, KC, 128], f8, name="w2s")
        wos = moew.tile([128, KC // 2, 2, D], f8, name="wos")

        GRP = [list(range(NC))]

        MARKS = []
        def mark(label):
            MARKS.append((label, nc.next_id()))
        nc._marks = MARKS

        # ---------------- helpers ----------------
        def rmsnorm(work, xt, out_bf, tagpfx=""):
            """out_bf = xt * rsqrt(mean(xt^2) + EPS), bf16."""
            ssum = work.tile([128, 1], f32, tag=tagpfx + "ss", name="ssum", bufs=4)
            nc.scalar.activation(sqscr[:], xt[:], AF.Square, accum_out=ssum[:])
            sd = work.tile([128, 1], f32, tag=tagpfx + "ss", name="sd", bufs=4)
            nc.scalar.activation(sd[:], ssum[:], AF.Sqrt, bias=eps_t[:],
                                 scale=1.0 / D)
            rr = work.tile([128, 1], f32, tag=tagpfx + "ss", name="rr", bufs=4)
            nc.vector.reciprocal(rr[:], sd[:])
            nc.vector.tensor_scalar_mul(out_bf[:], xt[:], rr[:])

        def ffn_chunk(j):
            bj, hfj, qj = j // 4, (j // 2) % 2, j % 2
            # assemble normed+transposed hn for this chunk's 512 tokens
            hnT = moe.tile([128, KC, 512], f8, tag="hnT", name="hnT")
            for q4 in range(4):
                agrow = fwork.tile([128, D], bf16, tag="agrow", name="agrow")
                nc.scalar.dma_start(agrow[:],
                                    agout[j][q4 * 128:(q4 + 1) * 128, :])
                xrow = fwork.tile([128, D], bf16, tag="xrow", name="xrow")
                nc.sync.dma_start(xrow[:],
                                  x_v[bj, hfj, qj, 2 * q4:2 * q4 + 2, :, :])
                hxr = fwork.tile([128, D], f32, tag="hxr", name="hxr")
                nc.vector.tensor_tensor(hxr[:], agrow[:], xrow[:], op=OP.add)
                hhx = fwork.tile([128, D], bf16, tag="hhx", name="hhx")
                rmsnorm(fwork, hxr, hhx, tagpfx="f")
                for kc in range(KC):
                    pt = P((128, 128), bf16, tag="pt", bufs=2)
                    nc.tensor.transpose(pt[:], hhx[:, kc * 128:(kc + 1) * 128],
                                        cst["identb"][:])
                    if kc % 2 == 0:
                        nc.scalar.copy(hnT[:, kc, q4 * 128:(q4 + 1) * 128],
                                       pt[:])
                    else:
                        nc.vector.tensor_copy(
                            hnT[:, kc, q4 * 128:(q4 + 1) * 128], pt[:])
            # router on this chunk's 512 tokens
            plog = P((E, 512), f32, tag="ps")
            for kc in range(KC):
                nc.tensor.matmul(plog[:], cst["rw"][:, kc // 2, kc % 2, :],
                                 hnT[:, kc, :],
                                 start=(kc == 0), stop=(kc == KC - 1))
            lsb = fwork.tile([E, 512], f32, tag="lsb", name="lsb", bufs=1)
            nc.scalar.copy(lsb[:], plog[:])
            wgt = fwork.tile([128, 4], f32, tag="wgt", name="wgt")
            for q4 in range(4):
                ptr = P((128, E), f32, tag="pt", bufs=2)
                nc.tensor.transpose(ptr[:], lsb[:, q4 * 128:(q4 + 1) * 128],
                                    cst["identf"][0:E, 0:E])
                nt = fwork.tile([128, E], f32, tag="nt", name="nt")
                nc.sync.dma_start(nt[:],
                                  noise_v[bj, hfj, qj, 2 * q4:2 * q4 + 2, :, :])
                zt = fwork.tile([128, E], f32, tag="zt", name="zt")
                nc.vector.scalar_tensor_tensor(zt[:], ptr[:], 0.125, nt[:],
                                               op0=OP.mult, op1=OP.add)
                ez = fwork.tile([128, E], f32, tag="ez", name="ez")
                den = fwork.tile([128, 1], f32, tag="den", name="den")
                nc.scalar.activation(ez[:], zt[:], AF.Exp, accum_out=den[:])
                rd = fwork.tile([128, 1], f32, tag="rd", name="rd")
                nc.vector.reciprocal(rd[:], den[:])
                pet = fwork.tile([128, E], f32, tag="pet", name="pet")
                nc.vector.tensor_tensor(pet[:], ez[:], cst["sel"][:], op=OP.mult)
                peu = fwork.tile([128, 1], f32, tag="peu", name="peu")
                nc.vector.reduce_sum(peu[:], pet[:], axis=AX.X)
                gtt = fwork.tile([128, E], f32, tag="gtt", name="gtt")
                nc.vector.tensor_scalar(gtt[:], ez[:], peu[:], None, op0=OP.is_gt)
                cnt = fwork.tile([128, 1], f32, tag="cnt", name="cnt")
                nc.vector.reduce_sum(cnt[:], gtt[:], axis=AX.X)
                ind = fwork.tile([128, 1], f32, tag="ind", name="ind")
                nc.vector.tensor_single_scalar(ind[:], cnt[:], 1.5, op=OP.is_lt)
                pw = fwork.tile([128, 1], f32, tag="pw", name="pw")
                nc.vector.tensor_tensor(pw[:], peu[:], rd[:], op=OP.mult)
                nc.vector.scalar_tensor_tensor(wgt[:, q4:q4 + 1], pw[:],
                                               1.0 / 64.0, ind[:],
                                               op0=OP.mult, op1=OP.mult)
            # expert SwiGLU
            ht = moe.tile([128, KC, 512], f8, tag="ht", name="ht", bufs=1)
            for hc in range(KC):
                p1 = P(tag="ps")
                for t in range(KC // 2):
                    nc.tensor.matmul(p1[:], w1s[:, t, :, hc, :],
                                     hnT[:, 2 * t:2 * t + 2, :],
                                     start=(t == 0), stop=(t == KC // 2 - 1),
                                     perf_mode=DR)
                p2 = P(tag="ps")
                for t in range(KC // 2):
                    nc.tensor.matmul(p2[:], w2s[:, t, :, hc, :],
                                     hnT[:, 2 * t:2 * t + 2, :],
                                     start=(t == 0), stop=(t == KC // 2 - 1),
                                     perf_mode=DR)
                s1 = fwork.tile([128, 512], f32, tag="s1", name="s1")
                nc.scalar.activation(s1[:], p1[:], AF.Silu,
                                     bias=cst["b1h"][:, hc:hc + 1], scale=0.125)
                nc.vector.scalar_tensor_tensor(
                    ht[:, hc, :], p2[:], cst["b2h"][:, hc:hc + 1], s1[:],
                    op0=OP.add, op1=OP.mult)
            # out-projection + gate + scatter to global rows
            for q4 in range(4):
                ob = fwork.tile([128, D], f32, tag="ob", name="ob")
                for db in range(2):
                    dsl = slice(db * 512, (db + 1) * 512)
                    peo = P(tag="ps")
                    for t in range(KC // 2):
                        nc.tensor.matmul(
                            peo[:], ht[:, 2 * t:2 * t + 2,
                                       q4 * 128:(q4 + 1) * 128],
                            wos[:, t, :, dsl],
                            start=(t == 0), stop=(t == KC // 2 - 1),
                            perf_mode=DR)
                    nc.vector.tensor_scalar_mul(ob[:, dsl], peo[:],
                                                wgt[:, q4:q4 + 1])
                gb0 = bj * 2048 + hfj * 1024 + (2 * q4) * 128 + qj * 64
                gb1 = gb0 + 128
                nc.sync.dma_start(io["out"][gb0:gb0 + 64, :], ob[0:64, :])
                nc.sync.dma_start(io["out"][gb1:gb1 + 64, :], ob[64:128, :])

        # =================================================================
        # Phase A: attention (TP over heads) with streamed RS -> AG
        # =================================================================
        with tc.tile_pool(name="apool", bufs=1) as apool, \
             tc.tile_pool(name="awork", bufs=2) as work, \
             tc.tile_pool(name="xnt", bufs=1) as xnt_pool, \
             tc.tile_pool(name="vsp", bufs=16) as vsp, \
             tc.tile_pool(name="expp", bufs=2) as expp:
            cst["masks"] = apool.tile([128, 4, 512], f32, name="masks")
            for nm in ("wq", "wk", "wv"):
                cst[nm] = apool.tile([128, KC, 128], bf16, name=nm)
            wo_t = [apool.tile([64, D], bf16, name=f"wo{h}") for h in range(2)]
            def load_qkv_consts():
                for nm in ("wq", "wk", "wv"):
                    nc.sync.dma_start(
                        cst[nm][:], io[nm].rearrange("(kc p) m -> p kc m", p=128))
            def load_attn_consts():
                nc.sync.dma_start(cst["masks"][:], io["masks"][:])
                nc.sync.dma_start(wo_t[0][:], io["wo0"][:])
                nc.sync.dma_start(wo_t[1][:], io["wo1"][:])
            qb = apool.tile([128, S], bf16, name="qb")
            kb = apool.tile([128, S], bf16, name="kb")
            vb = apool.tile([128, S], bf16, name="vb")
            oT = [apool.tile([66, S], bf16, name=f"oT{h}") for h in range(2)]
            rcp = [apool.tile([128, 16], f32, name=f"rcp{h}") for h in range(2)]
            vs_all = {}

            def qkv_block(b, sb):
                xnT = xnt_pool.tile([128, KC, 512], bf16, tag="xnT", name="xnT")
                for q4 in range(4):
                    st = b * 16 + sb * 4 + q4
                    xt = work.tile([128, D], f32, tag="xt", name="xt")
                    nc.sync.dma_start(xt[:], io["x"][st * 128:(st + 1) * 128, :])
                    xh = work.tile([128, D], bf16, tag="xh", name="xh")
                    rmsnorm(work, xt, xh)
                    for kc in range(KC):
                        pt = P((128, 128), bf16, tag="pt", bufs=2)
                        nc.tensor.transpose(pt[:], xh[:, kc * 128:(kc + 1) * 128],
                                            cst["identb"][:])
                        if kc % 2 == 0:
                            nc.scalar.copy(xnT[:, kc, q4 * 128:(q4 + 1) * 128],
                                           pt[:])
                        else:
                            nc.vector.tensor_copy(
                                xnT[:, kc, q4 * 128:(q4 + 1) * 128], pt[:])
                sl = slice(sb * 512, (sb + 1) * 512)
                ct = work.tile([128, 512], f32, tag="cos", name="ct")
                nc.sync.dma_start(ct[:], io["cosb"][:, sl])
                snt = work.tile([128, 512], f32, tag="sin", name="snt")
                nc.sync.dma_start(snt[:], io["sinb"][:, sl])
                for wname, dstT in (("wq", qb), ("wk", kb)):
                    pa = P(tag="ps")
                    for kc in range(KC):
                        nc.tensor.matmul(pa[:], cst[wname][:, kc, :],
                                         xnT[:, kc, :],
                                         start=(kc == 0), stop=(kc == KC - 1))
                    qs = work.tile([128, 512], f32, tag="qs", name="qs", bufs=1)
                    nc.vector.stream_shuffle(qs[:], pa[:], SHUF)
                    t1 = work.tile([128, 512], bf16, tag="t1", name="t1", bufs=1)
                    nc.vector.tensor_tensor(t1[:], pa[:], ct[:], op=OP.mult)
                    t2 = work.tile([128, 512], bf16, tag="t2", name="t2", bufs=1)
                    nc.gpsimd.tensor_tensor(t2[:], qs[:], snt[:], op=OP.mult)
                    nc.gpsimd.tensor_tensor(dstT[:, sl], t1[:], t2[:], op=OP.add)
                pv = P(tag="ps")
                for kc in range(KC):
                    nc.tensor.matmul(pv[:], cst["wv"][:, kc, :], xnT[:, kc, :],
                                     start=(kc == 0), stop=(kc == KC - 1))
                nc.scalar.copy(vb[:, sl], pv[:])

            def att_half(b, half):
                for h in range(2):
                    hr = slice(h * 64, (h + 1) * 64)
                    idn = (cst["identb"][0:64, 0:64] if h == 0
                           else cst["id64b"][64:128, 0:64])
                    for m in range(half * 8, half * 8 + 8):
                        gk = slice(m * 128, (m + 1) * 128)
                        ptv = P((128, 64), bf16, tag="pt", bufs=2)
                        nc.tensor.transpose(ptv[:], vb[hr, gk], idn)
                        vs = vsp.tile([128, 66], bf16, tag=f"vs{h}", name="vs")
                        if m % 2 == 0:
                            nc.scalar.copy(vs[:, 0:64], ptv[:])
                        else:
                            nc.vector.tensor_copy(vs[:, 0:64], ptv[:])
                        nc.vector.tensor_copy(vs[:, 64:66], vtail[:])
                        vs_all[(h, m)] = vs
                    for jq in (2 * half, 2 * half + 1):
                        gq = slice(jq * 512, (jq + 1) * 512)
                        pos = P(tag="pos", bufs=2)
                        nch = 4 * jq + 4
                        for m in range(nch):
                            gk = slice(m * 128, (m + 1) * 128)
                            pse = P(tag="ps")
                            nc.tensor.matmul(pse[:], kb[hr, gk], qb[hr, gq],
                                             start=True, stop=True)
                            if m >= 4 * jq:
                                nc.vector.tensor_tensor(
                                    pse[:], pse[:],
                                    cst["masks"][:, m - 4 * jq, :], op=OP.add)
                            et = expp.tile([128, 512], bf16, tag="et", name="et")
                            nc.scalar.activation(et[:], pse[:], AF.Exp,
                                                 scale=0.125)
                            nc.tensor.matmul(pos[0:66, :], vs_all[(h, m)][:],
                                             et[:], start=(m == 0),
                                             stop=(m == nch - 1))
                        nc.scalar.copy(oT[h][:, gq], pos[0:66, :])
                        for qt in range(4):
                            st16 = jq * 4 + qt
                            g1 = slice(jq * 512 + qt * 128,
                                       jq * 512 + (qt + 1) * 128)
                            pd = P((128, 1), bf16, tag="pt", bufs=2)
                            nc.tensor.transpose(pd[:], oT[h][64:65, g1],
                                                cst["identb"][64:65, 64:65])
                            nc.vector.reciprocal(rcp[h][:, st16:st16 + 1], pd[:])

            def outproj_half(b, half):
                r = b * 2 + half
                for k16 in range(8):
                    st16 = half * 8 + k16
                    g = slice(st16 * 128, (st16 + 1) * 128)
                    for db in range(2):
                        dsl = slice(db * 512, (db + 1) * 512)
                        pp0 = P(tag="ps")
                        nc.tensor.matmul(pp0[:], oT[0][0:64, g], wo_t[0][:, dsl],
                                         start=True, stop=True)
                        pp1 = P(tag="ps")
                        nc.tensor.matmul(pp1[:], oT[1][0:64, g], wo_t[1][:, dsl],
                                         start=True, stop=True)
                        abt = work.tile([128, 512], f32, tag="ab", name="abt",
                                        bufs=1)
                        nc.scalar.activation(abt[:], pp0[:], AF.Copy,
                                             scale=rcp[0][:, st16:st16 + 1])
                        ab2 = work.tile([128, 512], bf16, tag="ab2", name="ab2")
                        nc.vector.scalar_tensor_tensor(
                            ab2[:], pp1[:], rcp[1][:, st16:st16 + 1], abt[:],
                            op0=OP.mult, op1=OP.add)
                        nc.gpsimd.dma_start(
                            arq[r][k16 * 128:(k16 + 1) * 128, dsl], ab2[:])
                nc.gpsimd.collective_compute(
                    "ReduceScatter", OP.add, replica_groups=GRP,
                    ins=[arq[r][:]], outs=[rsq[r][:]])
                # shard -> h_out + immediate AllGathers (no compute between)
                nc.gpsimd.dma_start(io["h_out"][r * 128:(r + 1) * 128, :],
                                    rsq[r][:])
                for qj in range(2):
                    nc.gpsimd.collective_compute(
                        "AllGather", OP.bypass, replica_groups=GRP,
                        ins=[rsq[r][qj * 64:(qj + 1) * 64, :]],
                        outs=[agout[2 * r + qj][:]])

            # ---- batch 0 ----
            mark("qkv_b0")
            load_qkv_consts()
            qkv_block(0, 0)
            load_attn_consts()
            load_late_consts()
            qkv_block(0, 1)
            qkv_block(0, 2)
            qkv_block(0, 3)
            mark("att(0,0)")
            att_half(0, 0)
            mark("outproj(0,0)")
            outproj_half(0, 0)        # RS_0, AG_0, AG_1
            mark("att(0,1)")
            att_half(0, 1)
            mark("outproj(0,1)")
            outproj_half(0, 1)        # RS_1, AG_2, AG_3
            # FFN weights: transfer overlaps batch-1 attention
            nc.sync.dma_start(w1s[:], io["w1h"][:])
            nc.sync.dma_start(w2s[:], io["w2h"][:])
            nc.sync.dma_start(wos[:], io["wouth"][:])
            # ---- batch 1 ----
            mark("qkv_b1")
            for sb in range(4):
                qkv_block(1, sb)
            mark("att(1,0)")
            att_half(1, 0)
            mark("ffn0")
            ffn_chunk(0)
            mark("outproj(1,0)")
            outproj_half(1, 0)        # RS_2, AG_4, AG_5
            mark("att(1,1)")
            att_half(1, 1)
            mark("ffn1")
            ffn_chunk(1)
            mark("outproj(1,1)")
            outproj_half(1, 1)        # RS_3, AG_6, AG_7

        # =================================================================
        # Phase B: remaining expert FFN chunks
        # =================================================================
        for j in range(2, NCH):
            mark(f"ffn{j}")
            ffn_chunk(j)

    nc.compile()
    return nc


# =====================================================================
# Host-side input prep / output combine
# =====================================================================
def _bf16(a):
    import ml_dtypes
    return np.asarray(a, np.float32).astype(ml_dtypes.bfloat16)


def _f8(a):
    import ml_dtypes
    return np.asarray(a, np.float32).astype(ml_dtypes.float8_e4m3)


def prep_in_maps(inputs):
    x = np.asarray(inputs["x"], np.float32).reshape(T, D)
    scale1 = np.asarray(inputs["scale1"], np.float32)
    scale2 = np.asarray(inputs["scale2"], np.float32)
    wq = scale1[:, None] * np.asarray(inputs["wq"], np.float32)
    wk = scale1[:, None] * np.asarray(inputs["wk"], np.float32)
    wv = scale1[:, None] * np.asarray(inputs["wv"], np.float32)
    wo = np.asarray(inputs["wo"], np.float32)
    rw = scale2[:, None] * np.asarray(inputs["router_w"], np.float32)
    w1 = scale2[None, :, None] * np.asarray(inputs["w1"], np.float32)
    w2 = scale2[None, :, None] * np.asarray(inputs["w2"], np.float32)
    wout = np.asarray(inputs["wout"], np.float32)
    b1 = np.asarray(inputs["b1"], np.float32)
    b2 = np.asarray(inputs["b2"], np.float32)

    import jax
    noise = np.asarray(jax.random.gumbel(jax.random.key(42), (B, S, E),
                                         np.float32)) * 0.05
    noise = noise.reshape(T, E).astype(np.float32)

    # RoPE tables in the pair-interleaved layout:
    #   col r = hh*64 + 32*q2 + i      (i < 16)  -> x1 dim d = 16*q2 + i
    #   col r = hh*64 + 32*q2 + 16 + i           -> x2 dim d = 16*q2 + i
    half = DK // 2
    inv = 1.0 / (10000.0 ** (np.arange(half, dtype=np.float32) / half))
    ang = np.arange(S, dtype=np.float32)[:, None] * inv[None, :]  # [S, 32]
    cos_h, sin_h = np.cos(ang).T, np.sin(ang).T                   # [32, S]
    perm = np.zeros(128, np.int64)
    cosb = np.zeros((128, S), np.float32)
    sinb = np.zeros((128, S), np.float32)
    for hh in range(2):
        for q2 in range(2):
            for i in range(16):
                d = 16 * q2 + i
                r1 = hh * 64 + 32 * q2 + i
                r2 = r1 + 16
                perm[r1] = hh * 64 + d
                perm[r2] = hh * 64 + 32 + d
                cosb[r1] = cos_h[d]; cosb[r2] = cos_h[d]
                sinb[r1] = -sin_h[d]; sinb[r2] = sin_h[d]

    masks = np.zeros((128, 4, 512), np.float32)
    kr = np.arange(128)[:, None]
    qc = np.arange(512)[None, :]
    for t in range(4):
        masks[:, t, :] = np.where(kr + 128 * t <= qc, 0.0, MASKNEG)

    identb = np.eye(128, dtype=np.float32)
    id64b = np.zeros((128, 128), np.float32)
    id64b[64:128, 0:64] = np.eye(64, dtype=np.float32)

    xb = _bf16(x)
    in_maps = []
    for c in range(NC):
        cols = slice(c * 128, (c + 1) * 128)
        wq_c = _bf16(np.ascontiguousarray(wq[:, cols])[:, perm])
        wk_c = _bf16(np.ascontiguousarray(wk[:, cols])[:, perm])
        wv_c = _bf16(np.ascontiguousarray(wv[:, cols]))
        wo_c = np.ascontiguousarray(wo[cols, :])
        sel = np.zeros((128, E), np.float32)
        sel[:, c] = 1.0
        w1c = (8.0 * w1[c]).reshape(KC, 128, KC, 128).transpose(1, 0, 2, 3)
        w1c = w1c.reshape(128, KC // 2, 2, KC, 128)
        w2c = (8.0 * w2[c]).reshape(KC, 128, KC, 128).transpose(1, 0, 2, 3)
        w2c = w2c.reshape(128, KC // 2, 2, KC, 128)
        woc = (8.0 * wout[c]).reshape(KC, 128, D).transpose(1, 0, 2)
        woc = woc.reshape(128, KC // 2, 2, D)
        rwh = (8.0 * rw).reshape(KC, 128, E).transpose(1, 0, 2)
        rwh = rwh.reshape(128, KC // 2, 2, E)
        in_maps.append({
            "x": x, "xb": xb,
            "wq": wq_c, "wk": wk_c, "wv": wv_c,
            "wo0": _bf16(wo_c[0:64]), "wo1": _bf16(wo_c[64:128]),
            "cosb": cosb, "sinb": sinb, "masks": masks,
            "identb": _bf16(identb), "id64b": _bf16(id64b), "identf": identb,
            "rw": _f8(rwh), "noise": noise, "sel": sel,
            "w1h": _f8(w1c), "w2h": _f8(w2c), "wouth": _f8(woc),
            "b1h": np.ascontiguousarray(b1[c].reshape(KC, 128).T),
            "b2h": np.ascontiguousarray(8.0 * b2[c].reshape(KC, 128).T),
        })
    return in_maps


def combine(results):
    x = _COMB_X[0]
    h = np.array(x, np.float64)     # h = x + gathered attention sums
    for c in range(NC):
        hs = np.asarray(results[c]["h_out"], np.float32).astype(np.float64)
        for r in range(4):
            b, hf = r // 2, r % 2
            g0 = b * 2048 + hf * 1024 + c * 128
            h[g0:g0 + 128] += hs[r * 128:(r + 1) * 128]
    y = h
    for c in range(NC):
        y = y + results[c]["out"].astype(np.float64)
    return y.astype(np.float32).reshape(B, S, D)


_COMB_X = [None]


# ---------------------------------------------------------------------
# PJRT runner (axon): persistent jitted executable for the SPMD launch.
# ---------------------------------------------------------------------
import jax
from jax.sharding import Mesh, PartitionSpec
from jax.experimental.shard_map import shard_map
import concourse.mybir as mybir_mod
from concourse import bass2jax


def make_runner(nc, n_cores):
    bass2jax.install_neuronx_cc_hook()
    partition_name = nc.partition_id_tensor.name if nc.partition_id_tensor else None
    in_names, out_names, out_avals, zero_outs = [], [], [], []
    for alloc in nc.m.functions[0].allocations:
        if not isinstance(alloc, mybir_mod.MemoryLocationSet):
            continue
        name = alloc.memorylocations[0].name
        if alloc.kind == "ExternalInput":
            if name != partition_name:
                in_names.append(name)
        elif alloc.kind == "ExternalOutput":
            out_names.append(name)
            shape = tuple(alloc.tensor_shape)
            dtype = mybir_mod.dt.np(alloc.dtype)
            out_avals.append(jax.core.ShapedArray(shape, dtype))
            zero_outs.append(np.zeros(shape, dtype))
    n_params = len(in_names)
    n_outs = len(out_avals)
    all_in_names = list(in_names) + list(out_names)
    if partition_name is not None:
        all_in_names.append(partition_name)

    def _body(*args):
        operands = list(args)
        if partition_name is not None:
            operands.append(bass2jax.partition_id_tensor())
        outs = bass2jax._bass_exec_p.bind(
            *operands,
            out_avals=tuple(out_avals),
            in_names=tuple(all_in_names),
            out_names=tuple(out_names),
            lowering_input_output_aliases=(),
            sim_require_finite=True,
            sim_require_nnan=True,
            nc=nc,
        )
        return tuple(outs)

    devices = jax.devices()[:n_cores]
    mesh = Mesh(np.asarray(devices), ("core",))
    in_specs = (PartitionSpec("core"),) * (n_params + n_outs)
    out_specs = (PartitionSpec("core"),) * n_outs
    donate = tuple(range(n_params, n_params + n_outs))
    sharded = jax.jit(
        shard_map(_body, mesh=mesh, in_specs=in_specs, out_specs=out_specs,
                  check_rep=False),
        donate_argnums=donate, keep_unused=True,
    )

    def run(in_maps):
        per_core = [[np.asarray(m[name]) for name in in_names] for m in in_maps]
        concat_in = [np.concatenate([per_core[c][i] for c in range(n_cores)], axis=0)
                     for i in range(n_params)]
        concat_zeros = [np.zeros((n_cores * z.shape[0], *z.shape[1:]), z.dtype)
                        for z in zero_outs]
        out_arrs = sharded(*concat_in, *concat_zeros)
        out_arrs = [np.asarray(o) for o in out_arrs]
        return [
            {name: out_arrs[i].reshape(n_cores, *out_avals[i].shape)[c]
             for i, name in enumerate(out_names)}
            for c in range(n_cores)
        ]

    return run


_CACHE = {}


def kernel(**inputs):
    if "nc" not in _CACHE:
        _CACHE["nc"] = build_program()
        _CACHE["run"] = make_runner(_CACHE["nc"], NC)
    in_maps = prep_in_maps(inputs)
    _COMB_X[0] = np.asarray(inputs["x"], np.float32).reshape(T, D)
    results = _CACHE["run"](in_maps)
    return combine(results)
